# revision 15
# baseline (speedup 1.0000x reference)
"""Trainium2 Bass kernel for nn_AttnBlock (block-causal single-head attention
over video tokens, with RMS-norm and 1x1-conv q/k/v/out projections).

Shapes: x [2, 512, 8, 32, 32] -> S = 8*1024 = 8192 tokens per batch,
block-causal over frames (1024 tokens per frame).

Sharding: core = 4*b + ch handles batch b and the ch-th 256-query chunk of
EVERY frame -> all 8 cores run an identical instruction stream (SPMD) with
perfectly balanced block-causal attention work.

Per-core pipeline (matmuls bf16, fp32 PSUM accumulation):
  phase A: RMS scale r via ones-matmul sumsq -> sqrt -> recip -> outer-product
           broadcast matmul; hn = x*r (gamma folded into weights on host);
           K = Wk hn; V^T built directly as (hn-tile)^T @ Wv^T; Q = Wq hn.
  phase B: query frames processed in PAIRS (2j, 2j+1): shared key frames use
           N=512 matmuls covering both query blocks; the pair's extra frame
           uses N=256 for the odd block only. scoresT[k,q] -> exp on ACT from
           PSUM (scores are O(1): no max subtraction) -> PV + denominator
           accumulate in PSUM -> normalize -> Wo projection -> + residual
           (x_q + bo + Wo@bv, host-fused).
"""

import numpy as np
import ml_dtypes
from contextlib import ExitStack

# ---------------------------------------------------------------------------
# Walrus workaround: this container's walrus build accepts at most ONE sync
# wait command per instruction. Split excess waits onto same-engine NOPs
# (waits execute strictly earlier -> safe), including the Tile exit drain.
# ---------------------------------------------------------------------------
import bass_rust
import concourse.bass as bass
import concourse.mybir as mybir
import concourse.tile as tile
from concourse.vector_clock import ScopedClock
from concourse.bass_utils import run_bass_kernel_spmd

_MAX_WAITS = 1
_orig_lower = tile.TileContext._lower_ordered_insts


def _split_waits(nc, ordered):
    for bb, insts in ordered.items():
        out = []
        for inst in insts:
            si = inst.sync_info
            waits = list(si.on_wait) if si is not None and si.on_wait else []
            if (
                len(waits) > _MAX_WAITS
                and inst.engine is not None
                and inst.engine != mybir.EngineType.Unassigned
            ):
                for w in waits[:-_MAX_WAITS]:
                    out.append(
                        mybir.InstNoOp(
                            name=nc.get_next_instruction_name(),
                            engine=inst.engine,
                            bass_nofuse=True,
                            sync_info=mybir.SyncInfo(on_wait=[w], on_update=[]),
                        )
                    )
                si.on_wait = waits[-_MAX_WAITS:]
            out.append(inst)
        ordered[bb] = out


def _patched_lower(self, ordered):
    _split_waits(self.nc, ordered)
    return _orig_lower(self, ordered)


def _patched_drain_and_barrier(self, tick_clock, wait_clock):
    nc = self.nc
    drain_inst = nc.sync.drain()
    wait_clock.add_sem_waits(
        drain_inst.ins, ScopedClock({None: tick_clock.global_clock})
    )
    si = drain_inst.ins.sync_info
    waits = list(si.on_wait or []) if si is not None else []
    if len(waits) > _MAX_WAITS:
        si.on_wait = waits[:_MAX_WAITS]
        for i in range(_MAX_WAITS, len(waits), _MAX_WAITS):
            n = nc.sync.nop(nofuse=True)
            n.ins.sync_info = bass_rust.SyncInfo(
                on_wait=waits[i:i + _MAX_WAITS], on_update=[]
            )
    nc.all_engine_barrier()
    assert self.sems is not None
    popped = nc._tile_sem_poison_stack.pop()
    assert popped is self._sem_poison
    nc.clear_and_free_semaphores(list(self.sems.allocated().values()))
    nc.all_engine_barrier()


def _install_fix():
    tile.TileContext._lower_ordered_insts = _patched_lower
    tile.TileContext._drain_and_barrier = _patched_drain_and_barrier


# ---------------------------------------------------------------------------
# Problem constants (hardcoded per contract)
# ---------------------------------------------------------------------------
B, C, F, H, W = 2, 512, 8, 32, 32
HW = H * W            # 1024 tokens per frame
S = F * HW            # 8192 tokens per batch
P = 128
CT = C // P           # 4 channel tiles
QB = 256              # query block per frame per core
TQ = F * QB           # 2048 queries per core
CH = 512              # phase-A token chunk
NCH_K = S // CH       # 16
NCH_Q = TQ // CH      # 4
NKT = S // P          # 64 key tiles of 128
N_CORES = 8
APPROX_RECIP = False

f32 = mybir.dt.float32
f32r = mybir.dt.float32r
bf16 = mybir.dt.bfloat16
AF = mybir.ActivationFunctionType


def _build_nc():
    nc = bass.Bass("TRN2")

    xk = nc.dram_tensor("xk", [P, CT, S], f32, kind="ExternalInput")
    xq = nc.dram_tensor("xq", [P, CT, TQ], f32, kind="ExternalInput")
    xqres = nc.dram_tensor("xqres", [P, CT, F, QB], f32, kind="ExternalInput")
    wq_t = nc.dram_tensor("wq_t", [P, CT, C], bf16, kind="ExternalInput")
    wk_t = nc.dram_tensor("wk_t", [P, CT, C], bf16, kind="ExternalInput")
    wv_t = nc.dram_tensor("wv_t", [P, CT, C], bf16, kind="ExternalInput")
    wo_t = nc.dram_tensor("wo_t", [P, CT, C], bf16, kind="ExternalInput")
    b_qk = nc.dram_tensor("b_qk", [P, 2, CT], f32, kind="ExternalInput")
    out = nc.dram_tensor("out", [P, CT, F, QB], f32, kind="ExternalOutput")

    with tile.TileContext(nc) as tc, ExitStack() as ctx:
        big = ctx.enter_context(tc.tile_pool(name="big", bufs=1))
        K_sb = big.tile([P, CT, S], bf16)
        VT_sb = big.tile([P, NKT, C], bf16)
        Q_sb = big.tile([P, CT, TQ], bf16)

        const = ctx.enter_context(tc.tile_pool(name="const", bufs=1))
        ones_bf = const.tile([P, 1], bf16)
        nc.vector.memset(ones_bf, 1.0)
        ones_col = const.tile([1, P], f32)
        nc.vector.memset(ones_col, 1.0)
        bias_sb = const.tile([P, 2, CT], f32)
        nc.sync.dma_start(out=bias_sb, in_=b_qk[:, :, :])

        # ------------------------------------------------------------------
        # Phase A: norm + projections
        # ------------------------------------------------------------------
        with (
            tc.tile_pool(name="wA", bufs=1) as wA,
            tc.tile_pool(name="xload", bufs=3) as xload,
            tc.tile_pool(name="sq", bufs=2) as sqp,
            tc.tile_pool(name="hn", bufs=2) as hnp,
            tc.tile_pool(name="rsm", bufs=2) as rsm,
            tc.tile_pool(name="psA", bufs=3, space="PSUM") as psA,
            tc.tile_pool(name="psS", bufs=3, space="PSUM") as psS,
            tc.tile_pool(name="psRB", bufs=2, space="PSUM") as psRB,
        ):
            wk_sb = wA.tile([P, CT, C], bf16)
            nc.sync.dma_start(out=wk_sb, in_=wk_t[:, :, :])
            wv_sb = wA.tile([P, CT, C], bf16)
            nc.sync.dma_start(out=wv_sb, in_=wv_t[:, :, :])
            wq_sb = wA.tile([P, CT, C], bf16)
            nc.sync.dma_start(out=wq_sb, in_=wq_t[:, :, :])

            for src, nch, is_q in ((xk, NCH_K, False), (xq, NCH_Q, True)):
                for ck in range(nch):
                    sl = slice(ck * CH, (ck + 1) * CH)
                    # sumsq -> r chunk
                    ps_ss = psS.tile([1, CH], f32, tag="ss")
                    for ct in range(CT):
                        xt = xload.tile([P, CH], f32, tag="xt")
                        nc.sync.dma_start(out=xt, in_=src[:, ct, sl])
                        xsq = sqp.tile([P, CH], bf16, tag="xsq")
                        nc.vector.tensor_mul(xsq, xt, xt)
                        nc.tensor.matmul(
                            ps_ss, lhsT=ones_bf, rhs=xsq,
                            start=(ct == 0), stop=(ct == CT - 1),
                        )
                    rt = rsm.tile([1, CH], f32, tag="rt")
                    nc.scalar.activation(out=rt, in_=ps_ss, func=AF.Ln,
                                         scale=1.0 / C)
                    rr = rsm.tile([1, CH], f32, tag="rr")
                    nc.scalar.activation(out=rr, in_=rt, func=AF.Exp,
                                         scale=-0.5)
                    ps_rb = psRB.tile([P, CH], f32, tag="rb")
                    nc.tensor.matmul(ps_rb, lhsT=ones_col, rhs=rr,
                                     start=True, stop=True)
                    # hn = x * r  (bf16)
                    hn = hnp.tile([P, CT, CH], bf16, tag="hn")
                    for ct in range(CT):
                        xt = xload.tile([P, CH], f32, tag="xt2")
                        nc.sync.dma_start(out=xt, in_=src[:, ct, sl])
                        nc.vector.tensor_mul(hn[:, ct, :], xt, ps_rb)
                    if not is_q:
                        # K projection: [c_out, tok]
                        for co in range(CT):
                            pk = psA.tile([P, CH], f32, tag="proj")
                            for ci in range(CT):
                                nc.tensor.matmul(
                                    pk,
                                    lhsT=wk_sb[:, ci, co * P:(co + 1) * P],
                                    rhs=hn[:, ci, :],
                                    start=(ci == 0), stop=(ci == CT - 1),
                                )
                            nc.vector.tensor_scalar_add(
                                K_sb[:, co, sl], pk, bias_sb[:, 1, co:co + 1]
                            )
                        # V^T built directly: (hn k-tile)^T @ Wv^T -> [k, c]
                        for t in range(CH // P):
                            pv = psA.tile([P, C], f32, tag="proj")
                            for ci in range(CT):
                                nc.tensor.matmul(
                                    pv,
                                    lhsT=hn[:, ci, t * P:(t + 1) * P],
                                    rhs=wv_sb[:, ci, :],
                                    start=(ci == 0), stop=(ci == CT - 1),
                                )
                            kt = ck * (CH // P) + t
                            nc.scalar.copy(VT_sb[:, kt, :], pv)
                    else:
                        # Q projection
                        for co in range(CT):
                            pq = psA.tile([P, CH], f32, tag="proj")
                            for ci in range(CT):
                                nc.tensor.matmul(
                                    pq,
                                    lhsT=wq_sb[:, ci, co * P:(co + 1) * P],
                                    rhs=hn[:, ci, :],
                                    start=(ci == 0), stop=(ci == CT - 1),
                                )
                            nc.vector.tensor_scalar_add(
                                Q_sb[:, co, sl], pq, bias_sb[:, 0, co:co + 1]
                            )

        # ------------------------------------------------------------------
        # Phase B: block-causal attention, software-pipelined: PV lags the
        # score/exp stage by D tasks so the in-order PE never waits on ACT's
        # exp; pair finalization (normalize + Wo projection) is deferred two
        # tasks so its ACT/DVE chain overlaps the next pair's matmuls.
        # ------------------------------------------------------------------
        with (
            tc.tile_pool(name="wB", bufs=1) as wB,
            tc.tile_pool(name="xres", bufs=2) as xrp,
            tc.tile_pool(name="etp", bufs=4) as etp,
            tc.tile_pool(name="smp", bufs=2) as smp,
            tc.tile_pool(name="outp", bufs=3) as outp,
            tc.tile_pool(name="psbs", bufs=3, space="PSUM") as psbs,
            tc.tile_pool(name="psbo", bufs=1, space="PSUM") as psbo,
        ):
            wo_sb = wB.tile([P, CT, C], bf16)
            nc.sync.dma_start(out=wo_sb, in_=wo_t[:, :, :])

            inv_sqrt_c = 1.0 / float(np.sqrt(C))
            Q2 = 2 * QB

            tasks = []
            for j in range(F // 2):
                qa = 2 * j
                shared = (2 * j + 1) * (HW // P)
                nkt = shared + HW // P
                for kt in range(nkt):
                    ex = kt >= shared
                    tasks.append(dict(
                        j=j, kt=kt, qa=qa,
                        first=(kt == 0), last=(kt == nkt - 1),
                        qsl=(slice((qa + 1) * QB, (qa + 2) * QB) if ex
                             else slice(qa * QB, qa * QB + Q2)),
                        off=(QB if ex else 0), w=(QB if ex else Q2),
                    ))

            D = 2                      # PV lags scores by D tasks
            po_tiles = {}
            et_tiles = {}
            pair_state = {}
            fin1_due = {}              # i -> pair j: broadcast + on-muls
            fin2_due = {}              # i -> pair j: Wo projection + out

            def emit_scores_exp(i):
                t = tasks[i]
                if t["first"]:
                    po_tiles[t["j"]] = psbo.tile([P, 5, Q2], f32, tag="po",
                                                 name="po%d" % t["j"])
                w = t["w"]
                ps = psbs.tile([P, Q2], f32, tag="ps")
                for ct in range(CT):
                    nc.tensor.matmul(
                        ps[:, :w],
                        lhsT=K_sb[:, ct, t["kt"] * P:(t["kt"] + 1) * P],
                        rhs=Q_sb[:, ct, t["qsl"]],
                        start=(ct == 0), stop=(ct == CT - 1),
                    )
                et = etp.tile([P, Q2], bf16, tag="et")
                nc.scalar.activation(out=et[:, :w], in_=ps[:, :w],
                                     func=AF.Exp, scale=inv_sqrt_c)
                et_tiles[i] = et

            def emit_pv(i, cur_i):
                t = tasks[i]
                et = et_tiles.pop(i)
                po = po_tiles[t["j"]]
                w, off = t["w"], t["off"]
                for ct in range(CT):
                    nc.tensor.matmul(
                        po[:, ct, off:],
                        lhsT=VT_sb[:, t["kt"], ct * P:(ct + 1) * P],
                        rhs=et[:, :w],
                        start=t["first"], stop=t["last"],
                        skip_group_check=True,
                    )
                nc.tensor.matmul(
                    po[0:1, 4, off:], lhsT=ones_bf, rhs=et[:, :w],
                    start=t["first"], stop=t["last"], skip_group_check=True,
                )
                if t["last"]:
                    rd = smp.tile([1, Q2], f32, tag="rd")
                    nc.scalar.activation(out=rd, in_=po[0:1, 4, :], func=AF.Ln)
                    nc.scalar.activation(out=rd, in_=rd, func=AF.Exp,
                                         scale=-1.0)
                    pair_state[t["j"]] = rd
                    fin1_due[cur_i + 1] = t["j"]
                    fin2_due[cur_i + 2] = t["j"]

            def emit_fin1(j):
                po = po_tiles[j]
                rd = pair_state[j]
                nc.tensor.matmul(po[:, 4, :], lhsT=ones_col, rhs=rd,
                                 start=True, stop=True, skip_group_check=True)
                rb2 = smp.tile([P, Q2], f32, tag="rb2")
                nc.scalar.copy(rb2, po[:, 4, :])
                on = smp.tile([P, CT, Q2], bf16, tag="on")
                for ct in range(CT):
                    nc.vector.tensor_mul(on[:, ct, :], po[:, ct, :], rb2)
                pair_state[j] = on

            def emit_fin2(j):
                on = pair_state.pop(j)
                po_tiles.pop(j)
                qa = 2 * j
                for co in range(CT):
                    pf = psbs.tile([P, Q2], f32, tag="ps")
                    for ci in range(CT):
                        nc.tensor.matmul(
                            pf,
                            lhsT=wo_sb[:, ci, co * P:(co + 1) * P],
                            rhs=on[:, ci, :],
                            start=(ci == 0), stop=(ci == CT - 1),
                        )
                    xres_t = xrp.tile([P, Q2], f32, tag="xres")
                    nc.sync.dma_start(out=xres_t,
                                      in_=xqres[:, co, qa:qa + 2, :])
                    ot = outp.tile([P, Q2], f32, tag="ot")
                    nc.vector.tensor_add(ot, pf, xres_t)
                    nc.sync.dma_start(
                        out=out[:, co, qa:qa + 2, :],
                        in_=ot[:, :].rearrange("p (f t) -> p f t", t=QB),
                    )

            n = len(tasks)
            for i in range(n + D + 3):
                if i < n:
                    emit_scores_exp(i)
                if i in fin1_due:
                    emit_fin1(fin1_due.pop(i))
                if i in fin2_due:
                    emit_fin2(fin2_due.pop(i))
                if 0 <= i - D < n:
                    emit_pv(i - D, i)

    return nc


_NC = None


def _get_nc():
    global _NC
    if _NC is None:
        _install_fix()
        _NC = _build_nc()
    return _NC


def _to_pco(a):
    """[C, ...] -> [P, CT, ...] with channel c = ct*128 + p."""
    return np.ascontiguousarray(
        a.reshape(CT, P, *a.shape[1:]).swapaxes(0, 1)
    )


def kernel(x, gamma, wq, bq, wk, bk, wv, bv, wo, bo):
    x = np.asarray(x, dtype=np.float32)
    gamma = np.asarray(gamma, dtype=np.float32).reshape(C)
    wq, wk, wv, wo = (np.asarray(w, dtype=np.float32) for w in (wq, wk, wv, wo))
    bq, bk, bv, bo = (np.asarray(b, dtype=np.float32) for b in (bq, bk, bv, bo))

    # gamma folds into the input-channel scale of the q/k/v projections
    def prep_w(w, fold_gamma):
        wt = (w * gamma[None, :]).T if fold_gamma else w.T  # [c_in, c_out]
        return _to_pco(np.ascontiguousarray(wt)).astype(ml_dtypes.bfloat16)

    wq_t = prep_w(wq, True)
    wk_t = prep_w(wk, True)
    wv_t = prep_w(wv, True)
    wo_t = prep_w(wo, False)
    b_qk = np.stack([bq.reshape(CT, P), bk.reshape(CT, P)],
                    axis=0).transpose(2, 0, 1)
    b_qk = np.ascontiguousarray(b_qk)  # [P, 2, CT]

    # v-bias and out-bias fold into the residual: out = x + bo + Wo@bv + Wo@o0n
    res_bias = bo + wo @ bv  # [C]

    xf = x.reshape(B, C, F, HW)
    in_maps = []
    for core in range(N_CORES):
        b = core // 4
        ch = core % 4
        xk_full = _to_pco(xf[b].reshape(C, S))                    # [P, CT, S]
        xq_c = xf[b, :, :, ch * QB:(ch + 1) * QB]                 # [C, F, QB]
        xq_t = _to_pco(np.ascontiguousarray(xq_c).reshape(C, TQ)) # [P, CT, TQ]
        xqres = _to_pco(
            np.ascontiguousarray(xq_c + res_bias[:, None, None])
        )                                                         # [P, CT, F, QB]
        in_maps.append({
            "xk": xk_full, "xq": xq_t, "xqres": xqres,
            "wq_t": wq_t, "wk_t": wk_t, "wv_t": wv_t, "wo_t": wo_t,
            "b_qk": b_qk,
        })

    nc = _get_nc()
    res = run_bass_kernel_spmd(nc, in_maps, core_ids=list(range(N_CORES)))

    out = np.empty((B, C, F, HW), dtype=np.float32)
    for core in range(N_CORES):
        b = core // 4
        ch = core % 4
        o = res.results[core]["out"]              # [P, CT, F, QB]
        o = o.swapaxes(0, 1).reshape(C, F, QB)    # [C, F, QB]
        out[b, :, :, ch * QB:(ch + 1) * QB] = o
    return out.reshape(B, C, F, H, W)


# revision 16
# speedup vs baseline: 1.0212x; 1.0212x over previous
"""Trainium2 Bass kernel for nn_AttnBlock (block-causal single-head attention
over video tokens, with RMS-norm and 1x1-conv q/k/v/out projections).

Shapes: x [2, 512, 8, 32, 32] -> S = 8*1024 = 8192 tokens per batch,
block-causal over frames (1024 tokens per frame).

Sharding: core = 4*b + ch handles batch b and the ch-th 256-query chunk of
EVERY frame -> all 8 cores run an identical instruction stream (SPMD) with
perfectly balanced block-causal attention work.

Per-core pipeline (matmuls bf16, fp32 PSUM accumulation):
  phase A: RMS scale r via ones-matmul sumsq -> sqrt -> recip -> outer-product
           broadcast matmul; hn = x*r (gamma folded into weights on host);
           K = Wk hn; V^T built directly as (hn-tile)^T @ Wv^T; Q = Wq hn.
  phase B: query frames processed in PAIRS (2j, 2j+1): shared key frames use
           N=512 matmuls covering both query blocks; the pair's extra frame
           uses N=256 for the odd block only. scoresT[k,q] -> exp on ACT from
           PSUM (scores are O(1): no max subtraction) -> PV + denominator
           accumulate in PSUM -> normalize -> Wo projection -> + residual
           (x_q + bo + Wo@bv, host-fused).
"""

import numpy as np
import ml_dtypes
from contextlib import ExitStack

# ---------------------------------------------------------------------------
# Walrus workaround: this container's walrus build accepts at most ONE sync
# wait command per instruction. Split excess waits onto same-engine NOPs
# (waits execute strictly earlier -> safe), including the Tile exit drain.
# ---------------------------------------------------------------------------
import bass_rust
import concourse.bass as bass
import concourse.mybir as mybir
import concourse.tile as tile
from concourse.vector_clock import ScopedClock
from concourse.bass_utils import run_bass_kernel_spmd

_MAX_WAITS = 1
_orig_lower = tile.TileContext._lower_ordered_insts


def _split_waits(nc, ordered):
    for bb, insts in ordered.items():
        out = []
        for inst in insts:
            si = inst.sync_info
            waits = list(si.on_wait) if si is not None and si.on_wait else []
            if (
                len(waits) > _MAX_WAITS
                and inst.engine is not None
                and inst.engine != mybir.EngineType.Unassigned
            ):
                for w in waits[:-_MAX_WAITS]:
                    out.append(
                        mybir.InstNoOp(
                            name=nc.get_next_instruction_name(),
                            engine=inst.engine,
                            bass_nofuse=True,
                            sync_info=mybir.SyncInfo(on_wait=[w], on_update=[]),
                        )
                    )
                si.on_wait = waits[-_MAX_WAITS:]
            out.append(inst)
        ordered[bb] = out


def _patched_lower(self, ordered):
    _split_waits(self.nc, ordered)
    return _orig_lower(self, ordered)


def _patched_drain_and_barrier(self, tick_clock, wait_clock):
    nc = self.nc
    drain_inst = nc.sync.drain()
    wait_clock.add_sem_waits(
        drain_inst.ins, ScopedClock({None: tick_clock.global_clock})
    )
    si = drain_inst.ins.sync_info
    waits = list(si.on_wait or []) if si is not None else []
    if len(waits) > _MAX_WAITS:
        si.on_wait = waits[:_MAX_WAITS]
        for i in range(_MAX_WAITS, len(waits), _MAX_WAITS):
            n = nc.sync.nop(nofuse=True)
            n.ins.sync_info = bass_rust.SyncInfo(
                on_wait=waits[i:i + _MAX_WAITS], on_update=[]
            )
    nc.all_engine_barrier()
    assert self.sems is not None
    popped = nc._tile_sem_poison_stack.pop()
    assert popped is self._sem_poison
    nc.clear_and_free_semaphores(list(self.sems.allocated().values()))
    nc.all_engine_barrier()


def _install_fix():
    tile.TileContext._lower_ordered_insts = _patched_lower
    tile.TileContext._drain_and_barrier = _patched_drain_and_barrier


# ---------------------------------------------------------------------------
# Problem constants (hardcoded per contract)
# ---------------------------------------------------------------------------
B, C, F, H, W = 2, 512, 8, 32, 32
HW = H * W            # 1024 tokens per frame
S = F * HW            # 8192 tokens per batch
P = 128
CT = C // P           # 4 channel tiles
QB = 256              # query block per frame per core
TQ = F * QB           # 2048 queries per core
CH = 512              # phase-A token chunk
NCH_K = S // CH       # 16
NCH_Q = TQ // CH      # 4
NKT = S // P          # 64 key tiles of 128
N_CORES = 8
APPROX_RECIP = False

f32 = mybir.dt.float32
f32r = mybir.dt.float32r
bf16 = mybir.dt.bfloat16
AF = mybir.ActivationFunctionType


def _build_nc():
    nc = bass.Bass("TRN2")

    xk = nc.dram_tensor("xk", [P, CT, S], bf16, kind="ExternalInput")
    xq = nc.dram_tensor("xq", [P, CT, TQ], bf16, kind="ExternalInput")
    xqres = nc.dram_tensor("xqres", [P, CT, F, QB], f32, kind="ExternalInput")
    wq_t = nc.dram_tensor("wq_t", [P, CT, C], bf16, kind="ExternalInput")
    wk_t = nc.dram_tensor("wk_t", [P, CT, C], bf16, kind="ExternalInput")
    wv_t = nc.dram_tensor("wv_t", [P, CT, C], bf16, kind="ExternalInput")
    wo_t = nc.dram_tensor("wo_t", [P, CT, C], bf16, kind="ExternalInput")
    b_qk = nc.dram_tensor("b_qk", [P, 2, CT], f32, kind="ExternalInput")
    out = nc.dram_tensor("out", [P, CT, F, QB], f32, kind="ExternalOutput")

    with tile.TileContext(nc) as tc, ExitStack() as ctx:
        big = ctx.enter_context(tc.tile_pool(name="big", bufs=1))
        K_sb = big.tile([P, CT, S], bf16)
        VT_sb = big.tile([P, NKT, C], bf16)
        Q_sb = big.tile([P, CT, TQ], bf16)

        const = ctx.enter_context(tc.tile_pool(name="const", bufs=1))
        ones_bf = const.tile([P, 1], bf16)
        nc.vector.memset(ones_bf, 1.0)
        ones_col = const.tile([1, P], f32)
        nc.vector.memset(ones_col, 1.0)
        bias_sb = const.tile([P, 2, CT], f32)
        nc.sync.dma_start(out=bias_sb, in_=b_qk[:, :, :])

        # ------------------------------------------------------------------
        # Phase A: norm + projections
        # ------------------------------------------------------------------
        with (
            tc.tile_pool(name="wA", bufs=1) as wA,
            tc.tile_pool(name="xload", bufs=3) as xload,
            tc.tile_pool(name="sq", bufs=2) as sqp,
            tc.tile_pool(name="hn", bufs=2) as hnp,
            tc.tile_pool(name="rsm", bufs=2) as rsm,
            tc.tile_pool(name="psA", bufs=3, space="PSUM") as psA,
            tc.tile_pool(name="psS", bufs=3, space="PSUM") as psS,
            tc.tile_pool(name="psRB", bufs=2, space="PSUM") as psRB,
        ):
            wk_sb = wA.tile([P, CT, C], bf16)
            nc.sync.dma_start(out=wk_sb, in_=wk_t[:, :, :])
            wv_sb = wA.tile([P, CT, C], bf16)
            nc.sync.dma_start(out=wv_sb, in_=wv_t[:, :, :])
            wq_sb = wA.tile([P, CT, C], bf16)
            nc.sync.dma_start(out=wq_sb, in_=wq_t[:, :, :])

            for src, nch, is_q in ((xk, NCH_K, False), (xq, NCH_Q, True)):
                for ck in range(nch):
                    sl = slice(ck * CH, (ck + 1) * CH)
                    # sumsq -> r chunk
                    ps_ss = psS.tile([1, CH], f32, tag="ss")
                    for ct in range(CT):
                        xt = xload.tile([P, CH], bf16, tag="xt")
                        nc.sync.dma_start(out=xt, in_=src[:, ct, sl])
                        xsq = sqp.tile([P, CH], bf16, tag="xsq")
                        if ct % 2 == 0:
                            nc.vector.tensor_mul(xsq, xt, xt)
                        else:
                            nc.scalar.square(xsq, xt)
                        nc.tensor.matmul(
                            ps_ss, lhsT=ones_bf, rhs=xsq,
                            start=(ct == 0), stop=(ct == CT - 1),
                        )
                    rt = rsm.tile([1, CH], f32, tag="rt")
                    nc.scalar.activation(out=rt, in_=ps_ss, func=AF.Ln,
                                         scale=1.0 / C)
                    rr = rsm.tile([1, CH], f32, tag="rr")
                    nc.scalar.activation(out=rr, in_=rt, func=AF.Exp,
                                         scale=-0.5)
                    ps_rb = psRB.tile([P, CH], f32, tag="rb")
                    nc.tensor.matmul(ps_rb, lhsT=ones_col, rhs=rr,
                                     start=True, stop=True)
                    # hn = x * r  (bf16)
                    hn = hnp.tile([P, CT, CH], bf16, tag="hn")
                    for ct in range(CT):
                        xt = xload.tile([P, CH], bf16, tag="xt2")
                        nc.sync.dma_start(out=xt, in_=src[:, ct, sl])
                        nc.vector.tensor_mul(hn[:, ct, :], xt, ps_rb)
                    if not is_q:
                        # K projection: [c_out, tok]
                        for co in range(CT):
                            pk = psA.tile([P, CH], f32, tag="proj")
                            for ci in range(CT):
                                nc.tensor.matmul(
                                    pk,
                                    lhsT=wk_sb[:, ci, co * P:(co + 1) * P],
                                    rhs=hn[:, ci, :],
                                    start=(ci == 0), stop=(ci == CT - 1),
                                )
                            nc.vector.tensor_scalar_add(
                                K_sb[:, co, sl], pk, bias_sb[:, 1, co:co + 1]
                            )
                        # V^T built directly: (hn k-tile)^T @ Wv^T -> [k, c]
                        for t in range(CH // P):
                            pv = psA.tile([P, C], f32, tag="proj")
                            for ci in range(CT):
                                nc.tensor.matmul(
                                    pv,
                                    lhsT=hn[:, ci, t * P:(t + 1) * P],
                                    rhs=wv_sb[:, ci, :],
                                    start=(ci == 0), stop=(ci == CT - 1),
                                )
                            kt = ck * (CH // P) + t
                            nc.scalar.copy(VT_sb[:, kt, :], pv)
                    else:
                        # Q projection
                        for co in range(CT):
                            pq = psA.tile([P, CH], f32, tag="proj")
                            for ci in range(CT):
                                nc.tensor.matmul(
                                    pq,
                                    lhsT=wq_sb[:, ci, co * P:(co + 1) * P],
                                    rhs=hn[:, ci, :],
                                    start=(ci == 0), stop=(ci == CT - 1),
                                )
                            nc.vector.tensor_scalar_add(
                                Q_sb[:, co, sl], pq, bias_sb[:, 0, co:co + 1]
                            )

        # ------------------------------------------------------------------
        # Phase B: block-causal attention, software-pipelined: PV lags the
        # score/exp stage by D tasks so the in-order PE never waits on ACT's
        # exp; pair finalization (normalize + Wo projection) is deferred two
        # tasks so its ACT/DVE chain overlaps the next pair's matmuls.
        # ------------------------------------------------------------------
        with (
            tc.tile_pool(name="wB", bufs=1) as wB,
            tc.tile_pool(name="xres", bufs=2) as xrp,
            tc.tile_pool(name="etp", bufs=4) as etp,
            tc.tile_pool(name="smp", bufs=2) as smp,
            tc.tile_pool(name="outp", bufs=3) as outp,
            tc.tile_pool(name="psbs", bufs=3, space="PSUM") as psbs,
            tc.tile_pool(name="psbo", bufs=1, space="PSUM") as psbo,
        ):
            wo_sb = wB.tile([P, CT, C], bf16)
            nc.sync.dma_start(out=wo_sb, in_=wo_t[:, :, :])

            inv_sqrt_c = 1.0 / float(np.sqrt(C))
            Q2 = 2 * QB

            tasks = []
            for j in range(F // 2):
                qa = 2 * j
                shared = (2 * j + 1) * (HW // P)
                nkt = shared + HW // P
                for kt in range(nkt):
                    ex = kt >= shared
                    tasks.append(dict(
                        j=j, kt=kt, qa=qa,
                        first=(kt == 0), last=(kt == nkt - 1),
                        qsl=(slice((qa + 1) * QB, (qa + 2) * QB) if ex
                             else slice(qa * QB, qa * QB + Q2)),
                        off=(QB if ex else 0), w=(QB if ex else Q2),
                    ))

            D = 2                      # PV lags scores by D tasks
            po_tiles = {}
            et_tiles = {}
            pair_state = {}
            fin1_due = {}              # i -> pair j: broadcast + on-muls
            fin2_due = {}              # i -> pair j: Wo projection + out

            def emit_scores_exp(i):
                t = tasks[i]
                if t["first"]:
                    po_tiles[t["j"]] = psbo.tile([P, 5, Q2], f32, tag="po",
                                                 name="po%d" % t["j"])
                w = t["w"]
                ps = psbs.tile([P, Q2], f32, tag="ps")
                for ct in range(CT):
                    nc.tensor.matmul(
                        ps[:, :w],
                        lhsT=K_sb[:, ct, t["kt"] * P:(t["kt"] + 1) * P],
                        rhs=Q_sb[:, ct, t["qsl"]],
                        start=(ct == 0), stop=(ct == CT - 1),
                    )
                et = etp.tile([P, Q2], bf16, tag="et")
                nc.scalar.activation(out=et[:, :w], in_=ps[:, :w],
                                     func=AF.Exp, scale=inv_sqrt_c)
                et_tiles[i] = et

            def emit_pv(i, cur_i):
                t = tasks[i]
                et = et_tiles.pop(i)
                po = po_tiles[t["j"]]
                w, off = t["w"], t["off"]
                for ct in range(CT):
                    nc.tensor.matmul(
                        po[:, ct, off:],
                        lhsT=VT_sb[:, t["kt"], ct * P:(ct + 1) * P],
                        rhs=et[:, :w],
                        start=t["first"], stop=t["last"],
                        skip_group_check=True,
                    )
                nc.tensor.matmul(
                    po[0:1, 4, off:], lhsT=ones_bf, rhs=et[:, :w],
                    start=t["first"], stop=t["last"], skip_group_check=True,
                )
                if t["last"]:
                    rd = smp.tile([1, Q2], f32, tag="rd")
                    nc.scalar.activation(out=rd, in_=po[0:1, 4, :], func=AF.Ln)
                    nc.scalar.activation(out=rd, in_=rd, func=AF.Exp,
                                         scale=-1.0)
                    pair_state[t["j"]] = rd
                    fin1_due[cur_i + 1] = t["j"]
                    fin2_due[cur_i + 2] = t["j"]

            def emit_fin1(j):
                po = po_tiles[j]
                rd = pair_state[j]
                nc.tensor.matmul(po[:, 4, :], lhsT=ones_col, rhs=rd,
                                 start=True, stop=True, skip_group_check=True)
                rb2 = smp.tile([P, Q2], f32, tag="rb2")
                nc.scalar.copy(rb2, po[:, 4, :])
                on = smp.tile([P, CT, Q2], bf16, tag="on")
                for ct in range(CT):
                    nc.vector.tensor_mul(on[:, ct, :], po[:, ct, :], rb2)
                pair_state[j] = on

            def emit_fin2(j):
                on = pair_state.pop(j)
                po_tiles.pop(j)
                qa = 2 * j
                for co in range(CT):
                    pf = psbs.tile([P, Q2], f32, tag="ps")
                    for ci in range(CT):
                        nc.tensor.matmul(
                            pf,
                            lhsT=wo_sb[:, ci, co * P:(co + 1) * P],
                            rhs=on[:, ci, :],
                            start=(ci == 0), stop=(ci == CT - 1),
                        )
                    xres_t = xrp.tile([P, Q2], f32, tag="xres")
                    nc.sync.dma_start(out=xres_t,
                                      in_=xqres[:, co, qa:qa + 2, :])
                    ot = outp.tile([P, Q2], f32, tag="ot")
                    nc.vector.tensor_add(ot, pf, xres_t)
                    nc.sync.dma_start(
                        out=out[:, co, qa:qa + 2, :],
                        in_=ot[:, :].rearrange("p (f t) -> p f t", t=QB),
                    )

            n = len(tasks)
            for i in range(n + D + 3):
                if i < n:
                    emit_scores_exp(i)
                if i in fin1_due:
                    emit_fin1(fin1_due.pop(i))
                if i in fin2_due:
                    emit_fin2(fin2_due.pop(i))
                if 0 <= i - D < n:
                    emit_pv(i - D, i)

    return nc


_NC = None


def _get_nc():
    global _NC
    if _NC is None:
        _install_fix()
        _NC = _build_nc()
    return _NC


def _to_pco(a):
    """[C, ...] -> [P, CT, ...] with channel c = ct*128 + p."""
    return np.ascontiguousarray(
        a.reshape(CT, P, *a.shape[1:]).swapaxes(0, 1)
    )


def kernel(x, gamma, wq, bq, wk, bk, wv, bv, wo, bo):
    x = np.asarray(x, dtype=np.float32)
    gamma = np.asarray(gamma, dtype=np.float32).reshape(C)
    wq, wk, wv, wo = (np.asarray(w, dtype=np.float32) for w in (wq, wk, wv, wo))
    bq, bk, bv, bo = (np.asarray(b, dtype=np.float32) for b in (bq, bk, bv, bo))

    # gamma folds into the input-channel scale of the q/k/v projections
    def prep_w(w, fold_gamma):
        wt = (w * gamma[None, :]).T if fold_gamma else w.T  # [c_in, c_out]
        return _to_pco(np.ascontiguousarray(wt)).astype(ml_dtypes.bfloat16)

    wq_t = prep_w(wq, True)
    wk_t = prep_w(wk, True)
    wv_t = prep_w(wv, True)
    wo_t = prep_w(wo, False)
    b_qk = np.stack([bq.reshape(CT, P), bk.reshape(CT, P)],
                    axis=0).transpose(2, 0, 1)
    b_qk = np.ascontiguousarray(b_qk)  # [P, 2, CT]

    # v-bias and out-bias fold into the residual: out = x + bo + Wo@bv + Wo@o0n
    res_bias = bo + wo @ bv  # [C]

    xf = x.reshape(B, C, F, HW)
    in_maps = []
    for core in range(N_CORES):
        b = core // 4
        ch = core % 4
        xk_full = _to_pco(xf[b].reshape(C, S)).astype(ml_dtypes.bfloat16)
        xq_c = xf[b, :, :, ch * QB:(ch + 1) * QB]                 # [C, F, QB]
        xq_t = _to_pco(
            np.ascontiguousarray(xq_c).reshape(C, TQ)
        ).astype(ml_dtypes.bfloat16)
        xqres = _to_pco(
            np.ascontiguousarray(xq_c + res_bias[:, None, None])
        )                                                         # [P, CT, F, QB]
        in_maps.append({
            "xk": xk_full, "xq": xq_t, "xqres": xqres,
            "wq_t": wq_t, "wk_t": wk_t, "wv_t": wv_t, "wo_t": wo_t,
            "b_qk": b_qk,
        })

    nc = _get_nc()
    res = run_bass_kernel_spmd(nc, in_maps, core_ids=list(range(N_CORES)))

    out = np.empty((B, C, F, HW), dtype=np.float32)
    for core in range(N_CORES):
        b = core // 4
        ch = core % 4
        o = res.results[core]["out"]              # [P, CT, F, QB]
        o = o.swapaxes(0, 1).reshape(C, F, QB)    # [C, F, QB]
        out[b, :, :, ch * QB:(ch + 1) * QB] = o
    return out.reshape(B, C, F, H, W)
